# revision 6
# baseline (speedup 1.0000x reference)
"""Trainium2 Bass kernel for a sigmoid-scored attention decode step with KV cache.

Reference computation (all fp32):
    q = W_query @ x.T ; k = W_key @ x.T ; v = W_value @ x.T          # [4096, 1]
    K = [K_cache | k] ; V = [V_cache | v]                            # [4096, 8193]
    a = sigmoid((q.T @ K) / 64)                                      # [1, 8193]
    z = V @ a.T                                                      # [4096, 1]

Sharding: rows of all five matrices split across 8 NeuronCores (512 rows
each). Each core computes its q/k/v shard and partial scores over its 512
rows of K; partial score vectors are summed across cores via a hand-rolled
remote-DMA all-to-all; sigmoid + the V-weighted sum are local per shard.

v8 (vs v7 167us, v6 171us, v5 179us):
  - The ncfw warm-up AllReduce is GONE. Tile serializes every xbar
    transpose-DMA against other DMA-ish ops, and the collective counts: the
    a_x transpose (exchange critical path) waited for the collective, whose
    mesh cannot begin before ~80us into the NEFF (fixed firmware warm-up,
    measured across v5-v7). That pinned every variant to ~165us+. Launch
    skew without it measured ~4-6us on this runner (PJRT dispatch), which
    the all-to-all absorbs.
  - All big loads ride the sync HWDGE ring in priority order (wq, wk, kc,
    wvt, vct) — ring FIFO enforces arrival order, so V streams strictly
    after the score-critical tensors without a scheduler-proof gate. The
    scalar ring carries only the four xbar transpose-loads.
  - The 7 sends go over SWDGE queues 0-3 (num_swdge_queues=4), batched by
    peer distance (intra-die XOR 1-3 together, D2D pairs split), with
    per-queue triggers: ~4 sends in flight vs 7 serial. A remote SBUF write
    is 128 descriptors at ~0.3us (RMTV) / ~0.55us (D2D) non-posted
    round-trip each, so the all-to-all wall drops from ~36us to ~9us.
  - fp8-e3m4 W_q/W_k (x64), K_cache, W_v + bf16 V_cache flat images as in
    v6/v7; q/k/v rows on PE; PE pre-warmed during the first weight DMA.
"""

import sys

for _p in ("/opt/trn_rl_repo", "/root/.axon_site/_ro/trn_rl_repo"):
    if _p not in sys.path:
        sys.path.append(_p)

import ml_dtypes
import numpy as np

import concourse.bacc as bacc
import concourse.tile as tile
from concourse import mybir
from concourse.bass_utils import run_bass_kernel_spmd
from concourse.dve_ops import TENSOR_TENSOR_REDUCE

N_CORES = 8
E = 4096          # embedding dim (contraction for q/k/v)
D = 4096          # output dim
T = 8192          # cached timesteps
F32 = mybir.dt.float32
BF16 = mybir.dt.bfloat16
FP8 = mybir.dt.float8e3
BF16_NP = ml_dtypes.bfloat16
FP8_NP = ml_dtypes.float8_e3m4
WSCALE = 64.0     # host-side scale on W_q/W_k to lift them out of e3m4
                  # subnormals; folded back via the 1/64 column + act scale

# fp8 image column offsets (in elements)
WQT0 = 0          # W_q^T image  [128, 16384]
WKT0 = 16384      # W_k^T image
KC0 = 32768       # K_cache image [128, 32768], halves g=0,1 of 16384 cols
WVT0 = 65536      # W_v^T image
N8 = 81920
N16 = 32768       # bf16 image: V^T, halves of 16384 cols

# send j (to peer XOR j) -> SWDGE queue. XOR 1-3 are intra-die (fast wire),
# XOR 4-7 cross-die (slow): balance total wire time per queue.
SEND_QUEUE = {1: 0, 2: 1, 3: 1, 4: 2, 5: 2, 6: 3, 7: 3}


def build(n_cores=N_CORES, e=E, d_sh=D // N_CORES, t=T):
    wex = 80                     # exchange tile cols (>= 65, multiple of 16)
    payw = wex * 128             # staging row width (10240)

    nc = bacc.Bacc("TRN2", target_bir_lowering=False, debug=False,
                   num_devices=n_cores, num_swdge_queues=4)
    x_d = nc.dram_tensor("x", [1, e], BF16, kind="ExternalInput").ap()
    img8_d = nc.dram_tensor("img8", [128, N8], FP8, kind="ExternalInput").ap()
    img16_d = nc.dram_tensor("img16", [128, N16], BF16,
                             kind="ExternalInput").ap()
    z_d = nc.dram_tensor("z", [1, d_sh], F32, kind="ExternalOutput").ap()

    x_recv = nc.alloc_semaphore("x_recv")
    x_sent = nc.alloc_semaphore("x_sent")

    with tile.TileContext(nc) as tc:
        with (
            tc.tile_pool(name="keep", bufs=1) as kp,         # persistent tiles
            tc.tile_pool(name="dram", bufs=1, space="DRAM") as dramp,
        ):
            # --- x as interleaved chunk-columns: x_c[p, m] = x[128m + p] ---
            x_c = kp.tile([128, e // 128], BF16, tag="xc", name="x_c")
            nc.scalar.dma_start_transpose(
                x_c[:], x_d[0:1, :].rearrange("1 (r p) -> r p", p=128))

            # column of 1/64: folds the W-scale correction into the q.k
            # partition-sum matmul
            inv64_col = kp.tile([128, 1], BF16, tag="i64c", name="inv64_col")
            nc.vector.memset(inv64_col[:], 1.0 / WSCALE)
            # pre-warm the sigmoid ACT table off the critical path
            warm = kp.tile([1, 1], BF16, tag="warm", name="warm")
            nc.vector.memset(warm[:], 0.0)
            nc.scalar.activation(warm[:], warm[:],
                                 mybir.ActivationFunctionType.Sigmoid,
                                 scale=1.0 / 64.0)

            # partial-score staging row (bf16): [0,t) cache scores, t = q.k,
            # rest zero
            s_sb = kp.tile([1, payw], BF16, tag="s", name="s_sb")
            nc.vector.memset(s_sb[0:1, t + 1:payw], 0.0)

            # PE warm-up fodder
            dummy = kp.tile([128, 512], BF16, tag="dum", name="dummy")
            nc.vector.memset(dummy[:], 0.0)

            # --- sync-ring loads in priority order: ring FIFO guarantees the
            # --- score-critical tensors stream first ---
            wqt_t = [None, None]
            for hh in range(2):
                wqt_t[hh] = kp.tile([128, 8192], FP8, tag=f"wqt{hh}",
                                    name=f"wqt{hh}_t")
                nc.sync.dma_start(
                    wqt_t[hh][:], img8_d[:, WQT0 + 8192 * hh:WQT0 + 8192 * (hh + 1)])
            wkt_t = kp.tile([128, 16384], FP8, tag="wkt", name="wkt_t")
            nc.sync.dma_start(wkt_t[:], img8_d[:, WKT0:WKT0 + 16384])
            kc_t = [None, None]
            for g in range(2):
                kc_t[g] = kp.tile([128, 16384], FP8, tag=f"kc{g}",
                                  name=f"kc{g}_t")
                nc.sync.dma_start(
                    kc_t[g][:], img8_d[:, KC0 + 16384 * g:KC0 + 16384 * (g + 1)])
            wvt_t = kp.tile([128, 16384], FP8, tag="wvt", name="wvt_t")
            nc.sync.dma_start(wvt_t[:], img8_d[:, WVT0:WVT0 + 16384])
            vct_t = [None, None]
            for h in range(2):
                vct_t[h] = kp.tile([128, 16384], BF16, tag=f"vct{h}",
                                   name=f"vct{h}_t")
                nc.sync.dma_start(
                    vct_t[h][:], img16_d[:, 16384 * h:16384 * (h + 1)])

            psA_ctx = tc.tile_pool(name="psA", bufs=2, space="PSUM")
            psA = psA_ctx.__enter__()

            # --- PE warm-up: ~20 dummy matmuls ramp the p-state to 2.4GHz
            # --- while the first weight DMA is in flight ---
            wps = psA.tile([1, 512], F32, tag="psa", name="wps")
            for i in range(20):
                nc.tensor.matmul(wps[:], lhsT=inv64_col[:], rhs=dummy[:],
                                 start=True, stop=True)

            # --- q/k rows on PE: row = sum_m x_c[:,m]^T @ W^T[m-block] ---
            def w_row_matvec(w_tiles, name):
                # w_tiles: list of (tile, n_chunks) covering m-chunks in order
                ps = psA.tile([1, 512], F32, tag="psa", name=f"ps_{name}")
                m0 = 0
                total = sum(nch for _, nch in w_tiles)
                for w_tile, nch in w_tiles:
                    for mm in range(nch):
                        m = m0 + mm
                        nc.tensor.matmul(
                            ps[:], lhsT=x_c[:, m:m + 1],
                            rhs=w_tile[:, 512 * mm:512 * (mm + 1)],
                            start=(m == 0), stop=(m == total - 1))
                    m0 += nch
                row = kp.tile([1, 512], BF16, tag=f"row_{name}",
                              name=f"row_{name}")
                nc.vector.tensor_copy(row[:], ps[:])
                return row

            q_row = w_row_matvec([(wqt_t[0], 16), (wqt_t[1], 16)], "q")
            # bounce q through DRAM + xbar transpose into chunk-column layout
            q_dr = dramp.tile([1, 512], BF16, tag="qdr", name="q_dr")
            nc.gpsimd.dma_start(q_dr[:], q_row[:])
            q_x = kp.tile([128, 4], BF16, tag="qx", name="q_x")
            nc.scalar.dma_start_transpose(
                q_x[:], q_dr[0:1, :].rearrange("1 (r p) -> r p", p=128))

            k_row = w_row_matvec([(wkt_t, 32)], "k")
            k_dr = dramp.tile([1, 512], BF16, tag="kdr", name="k_dr")
            nc.gpsimd.dma_start(k_dr[:], k_row[:])
            k_x = kp.tile([128, 4], BF16, tag="kx", name="k_x")
            nc.scalar.dma_start_transpose(
                k_x[:], k_dr[0:1, :].rearrange("1 (r p) -> r p", p=128))

            psA_ctx.__exit__(None, None, None)

            psq_ctx = tc.tile_pool(name="psq", bufs=1, space="PSUM")
            psq = psq_ctx.__enter__()
            psp_ctx = tc.tile_pool(name="ps", bufs=3, space="PSUM")
            psp = psp_ctx.__enter__()
            psv_ctx = tc.tile_pool(name="psv", bufs=1, space="PSUM")
            psv = psv_ctx.__enter__()

            # --- partial scores: 8 psum tiles [1,1024], 4 chunk-matmuls x2 ---
            for g in range(2):
                for j in range(4):
                    ps = psp.tile([1, 1024], F32, tag="ps", name=f"ps{g}_{j}")
                    for c in range(4):
                        for m in range(2):
                            lo = 4096 * c + 1024 * j + 512 * m
                            nc.tensor.matmul(
                                ps[0:1, 512 * m:512 * (m + 1)],
                                lhsT=q_x[:, c:c + 1],
                                rhs=kc_t[g][:, lo:lo + 512],
                                start=(c == 0), stop=(c == 3),
                            )
                    # copy-cast f32 PSUM -> bf16 staging
                    nc.vector.tensor_copy(
                        s_sb[0:1, 4096 * g + 1024 * j:4096 * g + 1024 * (j + 1)],
                        ps[:])

            # --- appended score q.k/64 (the 1/64 column folds the extra
            # --- WSCALE^2 down to the same 64x the cache scores carry) ---
            qk_el = kp.tile([128, 4], BF16, tag="qkel", name="qk_el")
            qk_part = kp.tile([128, 1], BF16, tag="qkp", name="qk_part")
            nc.vector._custom_dve(
                TENSOR_TENSOR_REDUCE, out=qk_el[:], in0=q_x[:], in1=k_x[:],
                s0=0.0, s1=1.0, accum_out=qk_part[:],
            )
            qk_ps = psq.tile([1, 512], F32, tag="psq", name="qk_ps")
            nc.tensor.matmul(qk_ps[0:1, 0:1], lhsT=inv64_col[:],
                             rhs=qk_part[:], start=True, stop=True)
            nc.vector.tensor_copy(s_sb[0:1, t:t + 1], qk_ps[0:1, 0:1])

            # --- exchange: bounce to DRAM, transpose-load interleaved, send
            # --- to all 7 peers over SWDGE queues 0-3, reduce locally ---
            sc_d = dramp.tile([1, payw], BF16, tag="sc_d", name="sc_d")
            nc.gpsimd.dma_start(sc_d[:], s_sb[:])
            a_x = kp.tile([128, wex], BF16, tag="ax", name="a_x")
            nc.scalar.dma_start_transpose(
                a_x[:], sc_d[0:1, :].rearrange("1 (r p) -> r p", p=128))

            recvs = []
            for j in range(1, n_cores):
                rv = kp.tile([128, wex], BF16, tag=f"rv{j}", name=f"rv{j}")
                recvs.append(rv)
                rdests = [None] * 8
                rdests[j] = (0, j)
                nc.gpsimd.remote_dma_broadcast(
                    rv[:], a_x[:], remote_sem=x_recv, local_sem=x_sent,
                    rdests=rdests, queue_num=SEND_QUEUE[j],
                )
            for qn in range(4):
                nc.gpsimd.trigger_dma(count=None, queue_num=qn)

            # --- v row on PE while the exchange is in flight ---
            v_ps = psv.tile([1, 512], F32, tag="psv", name="v_ps")
            for m in range(32):
                nc.tensor.matmul(v_ps[:], lhsT=x_c[:, m:m + 1],
                                 rhs=wvt_t[:, 512 * m:512 * (m + 1)],
                                 start=(m == 0), stop=(m == 31))
            v_row = kp.tile([1, 512], BF16, tag="vrow", name="v_row")
            nc.vector.tensor_copy(v_row[:], v_ps[:])

            # accumulate the 7 peer contributions (ping-pong). The wait for
            # x_recv >= 14 is attached to the FIRST add post-scheduling (the
            # Tile scheduling simulator cannot model externally-satisfied
            # semaphores and would report a deadlock).
            asums = [kp.tile([128, wex], BF16, tag=f"asum{i}", name=f"asum{i}")
                     for i in range(2)]
            cur = a_x
            first_add = None
            for idx, rv in enumerate(recvs):
                nxt = asums[idx % 2]
                bi = nc.vector.scalar_tensor_tensor(
                    out=nxt[:], in0=cur[:], scalar=1.0, in1=rv[:],
                    op0=mybir.AluOpType.mult, op1=mybir.AluOpType.add,
                )
                if first_add is None:
                    first_add = bi
                cur = nxt
            a_fin = cur
            # staged scores carry WSCALE * (q.K); reference wants /64 inside
            # the sigmoid -> total scale 1/(WSCALE*64)
            nc.scalar.activation(a_fin[:, 0:65], a_fin[:, 0:65],
                                 mybir.ActivationFunctionType.Sigmoid,
                                 scale=1.0 / (WSCALE * 64.0))

            psv_ctx.__exit__(None, None, None)
            psp_ctx.__exit__(None, None, None)
            psq_ctx.__exit__(None, None, None)

            # --- z = V @ a on PE: 64 accumulating matmuls + appended column ---
            with tc.tile_pool(name="ps2", bufs=1, space="PSUM") as psp2:
                z_ps = psp2.tile([1, d_sh], F32, tag="zps", name="z_ps")
                for r in range(64):
                    h, rr = divmod(r, 32)
                    nc.tensor.matmul(
                        z_ps[:], lhsT=a_fin[:, r:r + 1],
                        rhs=vct_t[h][:, 512 * rr:512 * (rr + 1)],
                        start=(r == 0), stop=False,
                    )
                nc.tensor.matmul(z_ps[:], lhsT=a_fin[0:1, 64:65], rhs=v_row[:],
                                 start=False, stop=True)

                z_sb = kp.tile([1, d_sh], F32, tag="zsb", name="z_sb")
                nc.vector.tensor_copy(z_sb[:], z_ps[:])
                nc.sync.dma_start(z_d[:], z_sb[:])

    # Attach the external-semaphore wait to the first reduce add, now that
    # Tile scheduling is done: HW blocks the DVE queue here until all 7
    # remote writes have landed (each bumps x_recv by 2).
    ins = first_add.ins
    si = ins.sync_info
    ow = list(si.on_wait) if si is not None else []
    ou = list(si.on_update) if si is not None else []
    ow.append(mybir.SyncWait(sync_type="semaphore", id=x_recv.num,
                             wait_mode="sem-ge-imm",
                             wait_value=2 * (n_cores - 1)))
    ins.sync_info = mybir.SyncInfo(on_wait=ow, on_update=ou)

    nc.compile()
    return nc


def make_in_maps(inputs, n_cores=N_CORES, d_sh=D // N_CORES):
    x = np.ascontiguousarray(
        np.asarray(inputs["x"], np.float32).astype(BF16_NP))
    wq = np.asarray(inputs["W_query"], np.float32)
    wk = np.asarray(inputs["W_key"], np.float32)
    wv = np.asarray(inputs["W_value"], np.float32)
    kc = np.asarray(inputs["K_cache"], np.float32)
    vc = np.asarray(inputs["V_cache"], np.float32)

    def wt_img(w_sh, scale):
        # [512, e] shard -> W^T image [128, 32*512]:
        # img[p, 512m + d] = W^T[128m + p, d] = w_sh[d, 128m + p]
        wt = (w_sh * scale).T.astype(FP8_NP)          # [e, 512]
        return wt.reshape(32, 128, d_sh).transpose(1, 0, 2).reshape(128, -1)

    in_maps = []
    for i in range(n_cores):
        r0, r1 = d_sh * i, d_sh * (i + 1)
        kc_sh = kc[r0:r1].astype(FP8_NP)              # [512, 8192]
        # img[p, 16384g + 4096c + t'] = kc_sh[128c + p, 4096g + t']
        kc_img = (kc_sh.reshape(4, 128, 2, 4096)
                  .transpose(1, 2, 0, 3).reshape(128, -1))
        vct = vc[r0:r1].T.astype(BF16_NP)             # [8192, 512]
        # img[p, 512r + d] = V^T[128r + p, d]
        vct_img = (vct.reshape(64, 128, d_sh)
                   .transpose(1, 0, 2).reshape(128, -1))
        img8 = np.concatenate(
            [wt_img(wq[r0:r1], WSCALE), wt_img(wk[r0:r1], WSCALE),
             kc_img, wt_img(wv[r0:r1], 1.0)], axis=1)
        in_maps.append({
            "x": x,
            "img8": np.ascontiguousarray(img8),
            "img16": np.ascontiguousarray(vct_img),
        })
    return in_maps


def unshard(per_core_z, d_sh=D // N_CORES):
    shards = [np.asarray(zi)[0, :d_sh].reshape(d_sh, 1) for zi in per_core_z]
    return np.concatenate(shards, axis=0).astype(np.float32)


_NC_CACHE = None


def kernel(x, W_query, W_key, W_value, K_cache, V_cache):
    global _NC_CACHE
    if _NC_CACHE is None:
        _NC_CACHE = build()
    nc = _NC_CACHE
    in_maps = make_in_maps(dict(x=x, W_query=W_query, W_key=W_key,
                                W_value=W_value, K_cache=K_cache,
                                V_cache=V_cache))
    res = run_bass_kernel_spmd(nc, in_maps, core_ids=list(range(N_CORES)))
    return unshard([res.results[i]["z"] for i in range(N_CORES)])


# revision 12
# speedup vs baseline: 34.6444x; 34.6444x over previous
"""Trainium2 Bass kernel for a sigmoid-scored attention decode step with KV cache.

Reference computation (all fp32):
    q = W_query @ x.T ; k = W_key @ x.T ; v = W_value @ x.T          # [4096, 1]
    K = [K_cache | k] ; V = [V_cache | v]                            # [4096, 8193]
    a = sigmoid((q.T @ K) / 64)                                      # [1, 8193]
    z = V @ a.T                                                      # [4096, 1]

Sharding: rows of all five matrices split across 8 NeuronCores (512 rows
each). Each core computes its q/k/v shard and partial scores over its 512
rows of K; partial score vectors are summed across cores via a hand-rolled
remote-DMA all-to-all; sigmoid + the V-weighted sum are local per shard.

v8 (vs v7 167us, v6 171us, v5 179us):
  - The ncfw warm-up AllReduce must stay in the NEFF (without it NRT skips
    the synchronized multi-core launch and cores start milliseconds apart —
    measured 5.8ms), but everything about its placement changes. Tile
    serializes every xbar transpose-DMA against other DMA-ish ops including
    the collective, and the collective's mesh cannot begin before ~80us
    into the NEFF (fixed ncfw firmware warm-up, measured across v5-v7): in
    v6/v7 the a_x transpose (exchange critical path) therefore waited until
    ~92us. Now the collective is triggered AFTER the sends (every transpose
    precedes it in program order) and its output is published by the last
    gpsimd DMA, so the only thing the ~93us collective completion gates is
    the final teardown barrier. That makes ~98us the floor; everything else
    is arranged to finish by then.
  - Load split: sync ring takes wq halves, K-cache, V^T halves in FIFO
    priority order; scalar ring takes the transposes plus wk/wv (each
    sitting between the transposes so its data never delays one). The
    score-critical 8MiB streams first, V^T after.
  - The 7 sends go over SWDGE queues 0-3 (num_swdge_queues=4), batched by
    peer distance, with per-queue triggers: ~4 in flight vs 7 serial. A
    remote SBUF write is 128 descriptors at ~0.3us (RMTV) / ~0.55us (D2D)
    non-posted round-trip each, so the all-to-all wall drops ~36us -> ~10us.
  - fp8-e3m4 W_q/W_k (x64), K_cache, W_v + bf16 V_cache flat images as in
    v6/v7; q/k/v rows on PE; PE pre-warmed during the first weight DMA.
"""

import sys

for _p in ("/opt/trn_rl_repo", "/root/.axon_site/_ro/trn_rl_repo"):
    if _p not in sys.path:
        sys.path.append(_p)

import ml_dtypes
import numpy as np

import concourse.bacc as bacc
import concourse.tile as tile
from concourse import mybir
from concourse.bass_utils import run_bass_kernel_spmd
from concourse.dve_ops import TENSOR_TENSOR_REDUCE

N_CORES = 8
E = 4096          # embedding dim (contraction for q/k/v)
D = 4096          # output dim
T = 8192          # cached timesteps
F32 = mybir.dt.float32
BF16 = mybir.dt.bfloat16
FP8 = mybir.dt.float8e3
BF16_NP = ml_dtypes.bfloat16
FP8_NP = ml_dtypes.float8_e3m4
WSCALE = 64.0     # host-side scale on W_q/W_k to lift them out of e3m4
                  # subnormals; folded back via the 1/64 column + act scale

# fp8 image column offsets (in elements)
WQT0 = 0          # W_q^T image  [128, 16384]
WKT0 = 16384      # W_k^T image
KC0 = 32768       # K_cache image [128, 32768], halves g=0,1 of 16384 cols
WVT0 = 65536      # W_v^T image
N8 = 81920
N16 = 32768       # bf16 image: V^T, halves of 16384 cols

# send j (to peer XOR j) -> SWDGE queue. XOR 1-3 are intra-die (fast wire),
# XOR 4-7 cross-die (slow): balance total wire time per queue.
SEND_QUEUE = {1: 0, 2: 1, 3: 1, 4: 2, 5: 2, 6: 3, 7: 3}


def build(n_cores=N_CORES, e=E, d_sh=D // N_CORES, t=T):
    wex = 80                     # exchange tile cols (>= 65, multiple of 16)
    payw = wex * 128             # staging row width (10240)

    nc = bacc.Bacc("TRN2", target_bir_lowering=False, debug=False,
                   num_devices=n_cores, num_swdge_queues=4)
    x_d = nc.dram_tensor("x", [1, e], BF16, kind="ExternalInput").ap()
    img8_d = nc.dram_tensor("img8", [128, N8], FP8, kind="ExternalInput").ap()
    img16_d = nc.dram_tensor("img16", [128, N16], BF16,
                             kind="ExternalInput").ap()
    z_d = nc.dram_tensor("z", [1, d_sh], F32, kind="ExternalOutput").ap()
    cc_out_d = nc.dram_tensor("ccout", [1, 8], F32, kind="ExternalOutput").ap()

    x_recv = nc.alloc_semaphore("x_recv")
    x_sent = nc.alloc_semaphore("x_sent")

    with tile.TileContext(nc) as tc:
        with (
            tc.tile_pool(name="keep", bufs=1) as kp,         # persistent tiles
            tc.tile_pool(name="dram", bufs=1, space="DRAM") as dramp,
        ):
            # --- x as interleaved chunk-columns: x_c[p, m] = x[128m + p] ---
            x_c = kp.tile([128, e // 128], BF16, tag="xc", name="x_c")
            nc.scalar.dma_start_transpose(
                x_c[:], x_d[0:1, :].rearrange("1 (r p) -> r p", p=128))

            # warm-up collective input (the collective itself is triggered
            # after the sends, see below)
            cc_in_sb = kp.tile([1, 8], F32, tag="ccin_sb", name="cc_in_sb")
            nc.vector.memset(cc_in_sb[:], 0.0)
            cc_in_d = dramp.tile([1, 8], F32, tag="ccin", name="cc_in_d")
            cc_out_t = dramp.tile([1, 8], F32, tag="ccout_t", name="cc_out_t")
            nc.gpsimd.dma_start(cc_in_d[:], cc_in_sb[:])

            # column of 1/64: folds the W-scale correction into the q.k
            # partition-sum matmul
            inv64_col = kp.tile([128, 1], BF16, tag="i64c", name="inv64_col")
            nc.vector.memset(inv64_col[:], 1.0 / WSCALE)
            # pre-warm the sigmoid ACT table off the critical path
            warm = kp.tile([1, 1], BF16, tag="warm", name="warm")
            nc.vector.memset(warm[:], 0.0)
            nc.scalar.activation(warm[:], warm[:],
                                 mybir.ActivationFunctionType.Sigmoid,
                                 scale=1.0 / 64.0)

            # partial-score staging row (bf16): [0,t) cache scores, t = q.k,
            # rest zero
            s_sb = kp.tile([1, payw], BF16, tag="s", name="s_sb")
            nc.vector.memset(s_sb[0:1, t + 1:payw], 0.0)

            # PE warm-up fodder
            dummy = kp.tile([128, 512], BF16, tag="dum", name="dummy")
            nc.vector.memset(dummy[:], 0.0)

            # --- big loads, FIFO priority per ring. sync: wq halves, kc, V^T
            # --- halves. scalar: wk (after x_cT, before q_xT), wv (after
            # --- k_xT, before a_xT) — each big scalar-ring load sits where
            # --- its drain never delays a transpose that is ready. ---
            wqt_t = [None, None]
            for hh in range(2):
                wqt_t[hh] = kp.tile([128, 8192], FP8, tag=f"wqt{hh}",
                                    name=f"wqt{hh}_t")
                nc.sync.dma_start(
                    wqt_t[hh][:], img8_d[:, WQT0 + 8192 * hh:WQT0 + 8192 * (hh + 1)])
            wkt_t = kp.tile([128, 16384], FP8, tag="wkt", name="wkt_t")
            nc.scalar.dma_start(wkt_t[:], img8_d[:, WKT0:WKT0 + 16384])
            kc_t = [None, None]
            for g in range(2):
                kc_t[g] = kp.tile([128, 16384], FP8, tag=f"kc{g}",
                                  name=f"kc{g}_t")
                nc.sync.dma_start(
                    kc_t[g][:], img8_d[:, KC0 + 16384 * g:KC0 + 16384 * (g + 1)])
            vct_t = [None, None]
            for h in range(2):
                vct_t[h] = kp.tile([128, 16384], BF16, tag=f"vct{h}",
                                   name=f"vct{h}_t")
                nc.sync.dma_start(
                    vct_t[h][:], img16_d[:, 16384 * h:16384 * (h + 1)])

            psA_ctx = tc.tile_pool(name="psA", bufs=2, space="PSUM")
            psA = psA_ctx.__enter__()

            # --- PE warm-up: ~20 dummy matmuls ramp the p-state to 2.4GHz
            # --- while the first weight DMA is in flight ---
            wps = psA.tile([1, 512], F32, tag="psa", name="wps")
            for i in range(20):
                nc.tensor.matmul(wps[:], lhsT=inv64_col[:], rhs=dummy[:],
                                 start=True, stop=True)

            # --- q/k rows on PE: row = sum_m x_c[:,m]^T @ W^T[m-block] ---
            def w_row_matvec(w_tiles, name):
                # w_tiles: list of (tile, n_chunks) covering m-chunks in order
                ps = psA.tile([1, 512], F32, tag="psa", name=f"ps_{name}")
                m0 = 0
                total = sum(nch for _, nch in w_tiles)
                for w_tile, nch in w_tiles:
                    for mm in range(nch):
                        m = m0 + mm
                        nc.tensor.matmul(
                            ps[:], lhsT=x_c[:, m:m + 1],
                            rhs=w_tile[:, 512 * mm:512 * (mm + 1)],
                            start=(m == 0), stop=(m == total - 1))
                    m0 += nch
                row = kp.tile([1, 512], BF16, tag=f"row_{name}",
                              name=f"row_{name}")
                nc.vector.tensor_copy(row[:], ps[:])
                return row

            q_row = w_row_matvec([(wqt_t[0], 16), (wqt_t[1], 16)], "q")
            # bounce q through DRAM + xbar transpose into chunk-column layout
            q_dr = dramp.tile([1, 512], BF16, tag="qdr", name="q_dr")
            nc.gpsimd.dma_start(q_dr[:], q_row[:])
            q_x = kp.tile([128, 4], BF16, tag="qx", name="q_x")
            nc.scalar.dma_start_transpose(
                q_x[:], q_dr[0:1, :].rearrange("1 (r p) -> r p", p=128))

            k_row = w_row_matvec([(wkt_t, 32)], "k")
            k_dr = dramp.tile([1, 512], BF16, tag="kdr", name="k_dr")
            nc.gpsimd.dma_start(k_dr[:], k_row[:])
            k_x = kp.tile([128, 4], BF16, tag="kx", name="k_x")
            nc.scalar.dma_start_transpose(
                k_x[:], k_dr[0:1, :].rearrange("1 (r p) -> r p", p=128))

            # W_v^T on the scalar ring after k_xT: drains during the score
            # phase, well before the v row (~exchange time) needs it
            wvt_t = kp.tile([128, 16384], FP8, tag="wvt", name="wvt_t")
            nc.scalar.dma_start(wvt_t[:], img8_d[:, WVT0:WVT0 + 16384])

            psA_ctx.__exit__(None, None, None)

            psq_ctx = tc.tile_pool(name="psq", bufs=1, space="PSUM")
            psq = psq_ctx.__enter__()
            psp_ctx = tc.tile_pool(name="ps", bufs=3, space="PSUM")
            psp = psp_ctx.__enter__()
            psv_ctx = tc.tile_pool(name="psv", bufs=1, space="PSUM")
            psv = psv_ctx.__enter__()

            # --- partial scores: 8 psum tiles [1,1024], 4 chunk-matmuls x2 ---
            for g in range(2):
                for j in range(4):
                    ps = psp.tile([1, 1024], F32, tag="ps", name=f"ps{g}_{j}")
                    for c in range(4):
                        for m in range(2):
                            lo = 4096 * c + 1024 * j + 512 * m
                            nc.tensor.matmul(
                                ps[0:1, 512 * m:512 * (m + 1)],
                                lhsT=q_x[:, c:c + 1],
                                rhs=kc_t[g][:, lo:lo + 512],
                                start=(c == 0), stop=(c == 3),
                            )
                    # copy-cast f32 PSUM -> bf16 staging
                    nc.vector.tensor_copy(
                        s_sb[0:1, 4096 * g + 1024 * j:4096 * g + 1024 * (j + 1)],
                        ps[:])

            # --- appended score q.k/64 (the 1/64 column folds the extra
            # --- WSCALE^2 down to the same 64x the cache scores carry) ---
            qk_el = kp.tile([128, 4], BF16, tag="qkel", name="qk_el")
            qk_part = kp.tile([128, 1], BF16, tag="qkp", name="qk_part")
            nc.vector._custom_dve(
                TENSOR_TENSOR_REDUCE, out=qk_el[:], in0=q_x[:], in1=k_x[:],
                s0=0.0, s1=1.0, accum_out=qk_part[:],
            )
            qk_ps = psq.tile([1, 512], F32, tag="psq", name="qk_ps")
            nc.tensor.matmul(qk_ps[0:1, 0:1], lhsT=inv64_col[:],
                             rhs=qk_part[:], start=True, stop=True)
            nc.vector.tensor_copy(s_sb[0:1, t:t + 1], qk_ps[0:1, 0:1])

            # --- exchange: bounce to DRAM, transpose-load interleaved, send
            # --- to all 7 peers over SWDGE queues 0-3, reduce locally ---
            sc_d = dramp.tile([1, payw], BF16, tag="sc_d", name="sc_d")
            nc.gpsimd.dma_start(sc_d[:], s_sb[:])
            a_x = kp.tile([128, wex], BF16, tag="ax", name="a_x")
            nc.scalar.dma_start_transpose(
                a_x[:], sc_d[0:1, :].rearrange("1 (r p) -> r p", p=128))

            recvs = []
            for j in range(1, n_cores):
                rv = kp.tile([128, wex], BF16, tag=f"rv{j}", name=f"rv{j}")
                recvs.append(rv)
                rdests = [None] * 8
                rdests[j] = (0, j)
                nc.gpsimd.remote_dma_broadcast(
                    rv[:], a_x[:], remote_sem=x_recv, local_sem=x_sent,
                    rdests=rdests, queue_num=SEND_QUEUE[j],
                )
            for qn in range(4):
                nc.gpsimd.trigger_dma(count=None, queue_num=qn)

            # --- warm-up collective, AFTER the sends: every xbar transpose
            # --- precedes it in program order, so Tile's transpose-vs-DMA
            # --- serialization never puts the ~93us collective completion
            # --- on the exchange path. Publish is the last gpsimd op.
            nc.gpsimd.collective_compute(
                "AllReduce", mybir.AluOpType.add,
                replica_groups=[list(range(n_cores))],
                ins=[cc_in_d.opt()], outs=[cc_out_t.opt()],
            )
            nc.gpsimd.dma_start(cc_out_d, cc_out_t[:])

            # --- v row on PE while the exchange is in flight ---
            v_ps = psv.tile([1, 512], F32, tag="psv", name="v_ps")
            for m in range(32):
                nc.tensor.matmul(v_ps[:], lhsT=x_c[:, m:m + 1],
                                 rhs=wvt_t[:, 512 * m:512 * (m + 1)],
                                 start=(m == 0), stop=(m == 31))
            v_row = kp.tile([1, 512], BF16, tag="vrow", name="v_row")
            nc.vector.tensor_copy(v_row[:], v_ps[:])

            # accumulate the 7 peer contributions (ping-pong). The wait for
            # x_recv >= 14 is attached to the FIRST add post-scheduling (the
            # Tile scheduling simulator cannot model externally-satisfied
            # semaphores and would report a deadlock).
            asums = [kp.tile([128, wex], BF16, tag=f"asum{i}", name=f"asum{i}")
                     for i in range(2)]
            cur = a_x
            first_add = None
            for idx, rv in enumerate(recvs):
                nxt = asums[idx % 2]
                bi = nc.vector.scalar_tensor_tensor(
                    out=nxt[:], in0=cur[:], scalar=1.0, in1=rv[:],
                    op0=mybir.AluOpType.mult, op1=mybir.AluOpType.add,
                )
                if first_add is None:
                    first_add = bi
                cur = nxt
            a_fin = cur
            # staged scores carry WSCALE * (q.K); reference wants /64 inside
            # the sigmoid -> total scale 1/(WSCALE*64)
            nc.scalar.activation(a_fin[:, 0:65], a_fin[:, 0:65],
                                 mybir.ActivationFunctionType.Sigmoid,
                                 scale=1.0 / (WSCALE * 64.0))

            psv_ctx.__exit__(None, None, None)
            psp_ctx.__exit__(None, None, None)
            psq_ctx.__exit__(None, None, None)

            # --- z = V @ a on PE: 64 accumulating matmuls + appended column ---
            with tc.tile_pool(name="ps2", bufs=1, space="PSUM") as psp2:
                z_ps = psp2.tile([1, d_sh], F32, tag="zps", name="z_ps")
                for r in range(64):
                    h, rr = divmod(r, 32)
                    nc.tensor.matmul(
                        z_ps[:], lhsT=a_fin[:, r:r + 1],
                        rhs=vct_t[h][:, 512 * rr:512 * (rr + 1)],
                        start=(r == 0), stop=False,
                    )
                nc.tensor.matmul(z_ps[:], lhsT=a_fin[0:1, 64:65], rhs=v_row[:],
                                 start=False, stop=True)

                z_sb = kp.tile([1, d_sh], F32, tag="zsb", name="z_sb")
                nc.vector.tensor_copy(z_sb[:], z_ps[:])
                nc.sync.dma_start(z_d[:], z_sb[:])

    # Attach the external-semaphore wait to the first reduce add, now that
    # Tile scheduling is done: HW blocks the DVE queue here until all 7
    # remote writes have landed (each bumps x_recv by 2).
    ins = first_add.ins
    si = ins.sync_info
    ow = list(si.on_wait) if si is not None else []
    ou = list(si.on_update) if si is not None else []
    ow.append(mybir.SyncWait(sync_type="semaphore", id=x_recv.num,
                             wait_mode="sem-ge-imm",
                             wait_value=2 * (n_cores - 1)))
    ins.sync_info = mybir.SyncInfo(on_wait=ow, on_update=ou)

    nc.compile()
    return nc


def make_in_maps(inputs, n_cores=N_CORES, d_sh=D // N_CORES):
    x = np.ascontiguousarray(
        np.asarray(inputs["x"], np.float32).astype(BF16_NP))
    wq = np.asarray(inputs["W_query"], np.float32)
    wk = np.asarray(inputs["W_key"], np.float32)
    wv = np.asarray(inputs["W_value"], np.float32)
    kc = np.asarray(inputs["K_cache"], np.float32)
    vc = np.asarray(inputs["V_cache"], np.float32)

    def wt_img(w_sh, scale):
        # [512, e] shard -> W^T image [128, 32*512]:
        # img[p, 512m + d] = W^T[128m + p, d] = w_sh[d, 128m + p]
        wt = (w_sh * scale).T.astype(FP8_NP)          # [e, 512]
        return wt.reshape(32, 128, d_sh).transpose(1, 0, 2).reshape(128, -1)

    in_maps = []
    for i in range(n_cores):
        r0, r1 = d_sh * i, d_sh * (i + 1)
        kc_sh = kc[r0:r1].astype(FP8_NP)              # [512, 8192]
        # img[p, 16384g + 4096c + t'] = kc_sh[128c + p, 4096g + t']
        kc_img = (kc_sh.reshape(4, 128, 2, 4096)
                  .transpose(1, 2, 0, 3).reshape(128, -1))
        vct = vc[r0:r1].T.astype(BF16_NP)             # [8192, 512]
        # img[p, 512r + d] = V^T[128r + p, d]
        vct_img = (vct.reshape(64, 128, d_sh)
                   .transpose(1, 0, 2).reshape(128, -1))
        img8 = np.concatenate(
            [wt_img(wq[r0:r1], WSCALE), wt_img(wk[r0:r1], WSCALE),
             kc_img, wt_img(wv[r0:r1], 1.0)], axis=1)
        in_maps.append({
            "x": x,
            "img8": np.ascontiguousarray(img8),
            "img16": np.ascontiguousarray(vct_img),
        })
    return in_maps


def unshard(per_core_z, d_sh=D // N_CORES):
    shards = [np.asarray(zi)[0, :d_sh].reshape(d_sh, 1) for zi in per_core_z]
    return np.concatenate(shards, axis=0).astype(np.float32)


_NC_CACHE = None


def kernel(x, W_query, W_key, W_value, K_cache, V_cache):
    global _NC_CACHE
    if _NC_CACHE is None:
        _NC_CACHE = build()
    nc = _NC_CACHE
    in_maps = make_in_maps(dict(x=x, W_query=W_query, W_key=W_key,
                                W_value=W_value, K_cache=K_cache,
                                V_cache=V_cache))
    res = run_bass_kernel_spmd(nc, in_maps, core_ids=list(range(N_CORES)))
    return unshard([res.results[i]["z"] for i in range(N_CORES)])
